# revision 6
# baseline (speedup 1.0000x reference)
"""LNN / echo-state step on 8 TRN2 NeuronCores.

Computes state = 0.7*prev_state + 0.3*tanh(inputs @ Wi^T + prev_state @ Wr^T)
for B=8192, IN=2048, R=4096 (fp32).

Strategy: data-parallel over batch. Each of the 8 cores gets a 1024-row batch
shard and the full (replicated) weights, computes its shard's output with no
collectives, and the host reassembles.

Per-core kernel layout (all matmuls in float32r — fp32 bits read at FP22
precision, 1 cycle/row on the PE at N>=256, so bf16-rate with ~11-bit
mantissa accuracy):
  - out^T[r, b] accumulates over a fused contraction k in [0, 6144):
    k < 2048 contracts x^T against Wi^T tiles, k >= 2048 contracts h^T
    against Wr^T tiles. Activations (x^T and h^T, 24MB) stay resident in
    SBUF; weight tiles stream from HBM per output m-tile.
  - epilogue per [128, 512] tile: tanh on ScalarE from PSUM, then
    out = 0.7*h + 0.3*tanh on VectorE, DMA back to HBM.

Host-side numpy does the transposes/tiling so every DMA is contiguous.
"""

import numpy as np

import concourse.bass as bass
import concourse.mybir as mybir
from concourse import bacc
from concourse.tile import TileContext

P = 128
B_FULL, IN_DIM, R_DIM = 8192, 2048, 4096
N_CORES = 8
B_SHARD = B_FULL // N_CORES
LEAK = 0.3


def build_program(in_dim=IN_DIM, r_dim=R_DIM, b_shard=B_SHARD, ktc=4, n_tile=512):
    """Emit the per-core Bass program. Returns (nc, meta)."""
    kt_x = in_dim // P          # k-tiles from the input matmul
    kt_h = r_dim // P           # k-tiles from the reservoir matmul
    kt = kt_x + kt_h            # total fused contraction tiles
    mt = r_dim // P             # output row tiles (R on partitions)
    nt = b_shard // n_tile      # output column tiles
    nchunk = kt // ktc          # weight DMA chunks per m-tile
    assert kt % ktc == 0 and b_shard % n_tile == 0

    f32 = mybir.dt.float32
    f32r = mybir.dt.float32r
    Tanh = mybir.ActivationFunctionType.Tanh

    nc = bacc.Bacc("TRN2", target_bir_lowering=False, debug=False)

    acts_d = nc.dram_tensor("acts", [kt, P, b_shard], f32r, kind="ExternalInput")
    wts_d = nc.dram_tensor("wts", [mt, nchunk, P, ktc * P], f32r, kind="ExternalInput")
    out_d = nc.dram_tensor("out", [mt, P, b_shard], f32, kind="ExternalOutput")

    with TileContext(nc) as tc:
        with (
            tc.tile_pool(name="act_pool", bufs=kt) as apool,
            tc.tile_pool(name="w_pool", bufs=3) as wpool,
            tc.tile_pool(name="t_pool", bufs=2) as tpool,
            tc.tile_pool(name="o_pool", bufs=2) as opool,
            tc.tile_pool(name="ps_pool", bufs=4, space="PSUM") as pspool,
        ):
            act_tiles = []
            for k in range(kt):
                at = apool.tile([P, b_shard], f32r, tag="act", name=f"act{k}")
                nc.sync.dma_start(at[:], acts_d[k])
                act_tiles.append(at)

            for m in range(mt):
                psums = [pspool.tile([P, n_tile], f32, tag="ps", name=f"ps{m}_{n}")
                         for n in range(nt)]
                for ch in range(nchunk):
                    wc = wpool.tile([P, ktc * P], f32r, tag="w")
                    nc.sync.dma_start(wc[:], wts_d[m, ch])
                    for kl in range(ktc):
                        k = ch * ktc + kl
                        lhsT = wc[:, kl * P:(kl + 1) * P]
                        for n in range(nt):
                            rhs = act_tiles[k][:, n * n_tile:(n + 1) * n_tile]
                            nc.tensor.matmul(
                                psums[n][:],
                                lhsT,
                                rhs,
                                start=(k == 0),
                                stop=(k == kt - 1),
                            )
                for n in range(nt):
                    t = tpool.tile([P, n_tile], f32, tag="t")
                    nc.scalar.activation(t[:], psums[n][:], Tanh)
                    o = opool.tile([P, n_tile], f32, tag="o")
                    h_slice = act_tiles[kt_x + m][:, n * n_tile:(n + 1) * n_tile].bitcast(f32)
                    nc.vector.tensor_scalar_mul(o[:], h_slice, 1.0 - LEAK)
                    nc.vector.scalar_tensor_tensor(
                        o[:], t[:], LEAK, o[:],
                        mybir.AluOpType.mult, mybir.AluOpType.add,
                    )
                    nc.sync.dma_start(out_d[m, :, n * n_tile:(n + 1) * n_tile], o[:])

    nc.compile()
    meta = dict(in_dim=in_dim, r_dim=r_dim, b_shard=b_shard, ktc=ktc,
                n_tile=n_tile, kt_x=kt_x, kt_h=kt_h, kt=kt, mt=mt, nt=nt,
                nchunk=nchunk)
    return nc, meta


def pack_weights(input_weights, reservoir_weights, ktc=4):
    """[R, IN] + [R, R] fp32 -> [mt, nchunk, P, ktc*P] tiled for contiguous DMA."""
    w = np.concatenate(
        [np.ascontiguousarray(input_weights.T), np.ascontiguousarray(reservoir_weights.T)],
        axis=0,
    )  # [in+r, r]: w[k, r]
    k_dim, r_dim = w.shape
    kt, mt = k_dim // P, r_dim // P
    nchunk = kt // ktc
    w = w.reshape(nchunk, ktc, P, mt, P).transpose(3, 0, 2, 1, 4)
    return np.ascontiguousarray(w.reshape(mt, nchunk, P, ktc * P))


def pack_acts(x_shard, h_shard):
    """[b, in] + [b, r] fp32 -> [kt, P, b] (transposed, k-tiled)."""
    a = np.concatenate([x_shard.T, h_shard.T], axis=0)  # [in+r, b]
    k_dim, b = a.shape
    return np.ascontiguousarray(a.reshape(k_dim // P, P, b))


_CACHE = {}


def kernel(inputs, prev_state, input_weights, reservoir_weights):
    from concourse import bass_utils

    x = np.ascontiguousarray(np.asarray(inputs, dtype=np.float32))
    h = np.ascontiguousarray(np.asarray(prev_state, dtype=np.float32))
    wi = np.asarray(input_weights, dtype=np.float32)
    wr = np.asarray(reservoir_weights, dtype=np.float32)
    assert x.shape == (B_FULL, IN_DIM) and h.shape == (B_FULL, R_DIM)

    if "nc" not in _CACHE:
        _CACHE["nc"], _CACHE["meta"] = build_program()
    nc = _CACHE["nc"]

    wts = pack_weights(wi, wr)
    in_maps = []
    for c in range(N_CORES):
        sl = slice(c * B_SHARD, (c + 1) * B_SHARD)
        in_maps.append({"acts": pack_acts(x[sl], h[sl]), "wts": wts})

    res = bass_utils.run_bass_kernel_spmd(nc, in_maps, core_ids=list(range(N_CORES)))

    out = np.empty((B_FULL, R_DIM), dtype=np.float32)
    for c in range(N_CORES):
        o = res.results[c]["out"]  # [mt, P, b_shard]
        out[c * B_SHARD:(c + 1) * B_SHARD] = o.reshape(R_DIM, B_SHARD).T
    return out
